# revision 6
# baseline (speedup 1.0000x reference)
"""Trainium2 Bass kernel for CoarseDirectionReducer (segment_reduce).

out[b, n, g, :, :] = sum_j softmax(logits)[g, j] * x[b, n, GROUP_IDX[g, j], :, :]

Sharding: pure data parallel. The 16 (b, n) slices are split 2-per-core
across 8 NeuronCores; the tiny (8,3) softmax weights are computed on host
and baked into the kernel as immediates.

Per-core layout: each 256x256 channel plane is viewed as (128 partitions,
512), split into 2 half-plane chunks of 256 columns. One HWDGE DMA loads
all 24 channels of a chunk as a (128, 24, 256) SBUF tile (3 MB, 1 KB
contiguous segments per partition); each output group is one ACT
scaled-copy plus two fused DVE (x*w)+acc ops; one DMA stores (128, 8, 256).
"""

import numpy as np

import concourse.bass as bass
import concourse.mybir as mybir
from concourse.bass_utils import run_bass_kernel_spmd
from concourse.tile import TileContext
from concourse.vector_clock import ScopedClock, VectorClock


class SingleWaitTileContext(TileContext):
    """TileContext whose kernel-tail drain never carries more than one
    embedded sync wait.

    The walrus build in this container rejects instructions with more than
    one sync wait command. Tile's tail drain waits on every outstanding
    proc sem at once; split those into a chain of single-wait nops on the
    drain engine first, so the real drain has nothing left to wait on.
    """

    def _drain_and_barrier(self, tick_clock, wait_clock):
        gc = tick_clock.global_clock
        for proc in range(len(gc)):
            tick = gc[proc]
            if tick <= 0:
                continue
            nop = self.nc.sync.nop(nofuse=True, hint="drain_split")
            vc = VectorClock()
            vc.require_at_least(proc, tick)
            wait_clock.add_sem_waits(nop.ins, ScopedClock({None: vc}))
        # Same as TileContext._drain_and_barrier, but with no sem waits on
        # the drain itself: the nop chain above already made SP wait for
        # every outstanding proc, and the drain follows them in SP program
        # order.
        self.nc.sync.drain()
        self.nc.all_engine_barrier()
        assert self.sems is not None
        popped = self.nc._tile_sem_poison_stack.pop()
        assert popped is self._sem_poison
        self.nc.clear_and_free_semaphores(list(self.sems.allocated().values()))
        self.nc.all_engine_barrier()

N_CORES = 8
B, NCOARSE, NUM_FINE, H, W = 4, 4, 24, 256, 256
NGROUPS = 3  # members per group
NOUT = 8  # direction groups
SLICES = B * NCOARSE  # 16 (b, n) slices
SLICES_PER_CORE = SLICES // N_CORES  # 2
P = 128  # SBUF partitions; one plane = (128, 512)
PLANE_F = (H * W) // P  # 512
NH = 2  # half-plane chunks
F = PLANE_F // NH  # 256

GROUPS_DXDY = (((1, 1), (2, 2), (2, 1)), ((0, 1), (0, 2), (1, 2)),
               ((-1, 1), (-2, 2), (-1, 2)), ((1, 0), (2, 0), (2, -1)),
               ((-1, 0), (-2, 0), (-2, 1)), ((1, -1), (2, -2), (1, -2)),
               ((0, -1), (0, -2), (-1, -2)), ((-1, -1), (-2, -2), (-2, -1)))


def _group_indices():
    offsets_dydx = [(dy, dx) for dy in range(-2, 3) for dx in range(-2, 3)
                    if (dy, dx) != (0, 0)]
    off_to_idx = {(dx, dy): i for i, (dy, dx) in enumerate(offsets_dydx)}
    return np.array([[off_to_idx[o] for o in g] for g in GROUPS_DXDY],
                    dtype=np.int32)  # (8, 3)


GROUP_IDX = _group_indices()

_LAST_RESULT = None  # BassKernelResults of the most recent run (for test.py)


def build_nc(wg: np.ndarray) -> bass.Bass:
    """Build the per-core Bass program. wg: (8, 3) f32 softmax weights."""
    f32 = mybir.dt.float32
    mult = mybir.AluOpType.mult
    add = mybir.AluOpType.add
    copy_fn = mybir.ActivationFunctionType.Copy

    nc = bass.Bass()
    x = nc.declare_dram_parameter(
        "x", [SLICES_PER_CORE, NUM_FINE, P, NH, F], f32, isOutput=False)
    y = nc.declare_dram_parameter(
        "y", [SLICES_PER_CORE, NOUT, P, NH, F], f32, isOutput=True)

    with SingleWaitTileContext(nc) as tc:
        with (
            tc.tile_pool(name="xin", bufs=4) as xin_pool,
            tc.tile_pool(name="yout", bufs=4) as yout_pool,
        ):
            for s in range(SLICES_PER_CORE):
                for h in range(NH):
                    it = xin_pool.tile([P, NUM_FINE, F], f32)
                    nc.sync.dma_start(
                        it[:], x[s, :, :, h, :].rearrange("c p f -> p c f"))
                    ot = yout_pool.tile([P, NOUT, F], f32)
                    for g in range(NOUT):
                        i0, i1, i2 = (int(i) for i in GROUP_IDX[g])
                        w0, w1, w2 = (float(v) for v in wg[g])
                        nc.vector.tensor_scalar(
                            ot[:, g], it[:, i0], w0, None, mult)
                        nc.vector.scalar_tensor_tensor(
                            ot[:, g], it[:, i1], w1, ot[:, g], mult, add)
                        nc.vector.scalar_tensor_tensor(
                            ot[:, g], it[:, i2], w2, ot[:, g], mult, add)
                    nc.scalar.dma_start(
                        y[s, :, :, h, :].rearrange("g p f -> p g f"), ot[:])
    return nc


def _softmax_rows(logits: np.ndarray) -> np.ndarray:
    z = logits.astype(np.float32)
    z = z - z.max(axis=-1, keepdims=True)
    e = np.exp(z)
    return e / e.sum(axis=-1, keepdims=True)


def kernel(fine_features: np.ndarray, logits: np.ndarray) -> np.ndarray:
    global _LAST_RESULT
    ff = np.asarray(fine_features, dtype=np.float32)
    wg = _softmax_rows(np.asarray(logits, dtype=np.float32))

    # (B, 96, H, W) -> (16 slices, 24, 128, 2, 256); slicing the outer axis
    # keeps each core's shard a contiguous zero-copy view.
    x16 = ff.reshape(SLICES, NUM_FINE, P, NH, F)
    in_maps = [
        {"x": x16[SLICES_PER_CORE * k:SLICES_PER_CORE * (k + 1)]}
        for k in range(N_CORES)
    ]

    nc = build_nc(wg)
    res = run_bass_kernel_spmd(nc, in_maps, core_ids=list(range(N_CORES)))
    _LAST_RESULT = res

    out16 = np.concatenate([res.results[k]["y"] for k in range(N_CORES)],
                           axis=0)  # (16, 8, 128, 2, 256)
    return out16.reshape(B, NCOARSE * NOUT, H, W)
